# revision 22
# baseline (speedup 1.0000x reference)
"""Trainium2 Bass kernel for nn_DWT1D: single-level bior2.2 DWT (symmetric
mode, pywt convention) + linear downsample, fused.

    x [32, 64, 16384] f32  ->  out [32, 192, 8194] f32
    out[:, 0:64]    = linear interp of x to length L   (align_corners=False)
    out[:, 64:128]  = DWT approx coeffs
    out[:, 128:192] = DWT detail coeffs

Strategy: pure data parallel over batch (4 batches = 256 (b,c) rows per
NeuronCore x 8 cores).  Rows live on SBUF partitions (2 row-tiles of 128);
the signal axis is chunked.  The bior2.2 (CDF 5/3) filters are evaluated
with a lifting-style factorization on even/odd phases, read directly from
the interleaved input via stride-2 APs (fp32 tensor_tensor is 1x mode on
DVE regardless of stride, so deinterleaving would be pure overhead):

    D2[l] = x[2l-2] + x[2l] - 2*x[2l-1]         (DVE: TT add + STT)
    d[l]  = 0.35355339 * D2[l]                  (ACT scaled copy)
    A[l]  = x[2l-2] - 0.125*(D2[l-1] + D2[l])   (Pool add + DVE STT)
    a[l]  = sqrt(2) * A[l]                      (ACT scaled copy)

The interp is piecewise "i0 = 2l - s" with s constant on 6 runs; weights
are host-precomputed replicating the reference's f32 arithmetic exactly,
shipped as a [1, L] input and broadcast across partitions once via a K=1
PE matmul (ones^T @ w -> PSUM -> SBUF).

    ds[l] = x[i0] + w[l]*(x[i1] - x[i0])        (DVE sub + Pool mul + DVE add)

Symmetric-extension halos are materialized in the input tile with small
reversed-AP on-chip copies so every output column uses the interior
formula.  All DMAs are HWDGE (input on SP ring, output on ACT ring).
"""

import json

import numpy as np

# ---------------------------------------------------------------- constants
B, C, N = 32, 64, 16384
L = (N + 6 - 1) // 2  # 8194
NCORES = 8
BPC = B // NCORES  # batches per core
ROWS = BPC * C  # 256 rows per core
RT = ROWS // 128  # row-tiles per core
CK = 2048  # output-chunk length (last chunk 2050)
CKM = L - (L // CK - 1) * CK if L % CK else CK  # max chunk len
SQ2 = 1.4142135623730951
ISQ8 = 0.35355339059327378  # 1/(2*sqrt(2))

_MAX_WAITS = 1
_REAL_ENGINES = {"SP", "DVE", "PE", "Pool", "Activation"}


# ------------------------------------------------- BIR sync-wait splitting
def _split_sync_waits(bir_bytes, max_waits=_MAX_WAITS):
    """This neuronxcc build rejects instructions carrying more than one
    semaphore wait ("Too many sync wait commands").  Move excess waits onto
    same-engine NoOps inserted right before the instruction — an earlier
    wait on the same in-order engine is semantically identical."""
    d = json.loads(bir_bytes)
    ctr = 0
    changed = False
    for fn in d.get("functions", []):
        for bb in fn.get("blocks", []):
            out = []
            for inst in bb.get("instructions", []):
                si = inst.get("sync_info")
                ow = (si or {}).get("on_wait") or []
                if len(ow) > max_waits and inst.get("engine") in _REAL_ENGINES:
                    si["on_wait"] = ow[:max_waits]
                    for w in ow[max_waits:]:
                        ctr += 1
                        changed = True
                        out.append(
                            {
                                "debug": inst.get("debug", 0),
                                "engine": inst["engine"],
                                "ins": [],
                                "outs": [],
                                "name": f"{inst['name']}-wsp{ctr}",
                                "opcode": "NoOp",
                                "sync_info": {"on_update": [], "on_wait": [w]},
                            }
                        )
                out.append(inst)
            bb["instructions"] = out
    if not changed:
        return bir_bytes
    return json.dumps(d).encode()


_wait_patch_done = False


def _install_wait_splitter():
    global _wait_patch_done
    if _wait_patch_done:
        return
    _wait_patch_done = True
    import concourse.bass as bass

    orig = bass.Bass.to_json_bytes

    def to_json_bytes(self, *a, **k):
        return _split_sync_waits(orig(self, *a, **k))

    bass.Bass.to_json_bytes = to_json_bytes


# ------------------------------------------------------- interp host consts
def _interp_consts():
    """Replicate the reference's f32 arithmetic for the interp grid exactly:
    src = (arange(L) + 0.5) * f32(N/L) - 0.5 ; i0 = floor(src); w = src-i0."""
    ratio = np.float32(N / L)
    src = (np.arange(L, dtype=np.float32) + np.float32(0.5)) * ratio
    src = src - np.float32(0.5)
    src = np.clip(src, np.float32(0.0), np.float32(N - 1))
    i0 = np.floor(src).astype(np.int64)
    w = src - i0.astype(np.float32)
    ls = np.arange(L, dtype=np.int64)
    s = 2 * ls - i0
    # interior-formula preconditions (no clipping active, i1 = i0+1 in range)
    assert i0.min() >= 0 and i0.max() <= N - 2, (i0.min(), i0.max())
    assert s.min() >= 0
    return w.astype(np.float32), s


def _pieces_for_chunk(s, l0, l1):
    """Split [l0, l1) into maximal runs of constant s."""
    pieces = []
    a = l0
    while a < l1:
        b = a + 1
        while b < l1 and s[b] == s[a]:
            b += 1
        pieces.append((a, b, int(s[a])))
        a = b
    return pieces


def _chunks():
    starts = list(range(0, L - CKM + 1, CK))
    out = []
    for i, st in enumerate(starts):
        en = st + CK if i < len(starts) - 1 else L
        out.append((st, en))
    assert out[-1][1] == L
    return out


# ------------------------------------------------------------ program build
def _build_program(n_repeat=1):
    import concourse.bass as bass
    import concourse.mybir as mybir
    from concourse.tile import TileContext

    _install_wait_splitter()

    f32 = mybir.dt.float32
    MUL = mybir.AluOpType.mult
    ADD = mybir.AluOpType.add

    w32, s_arr = _interp_consts()

    nc = bass.Bass(dynamic_dma_scratch_size=32768)
    x_d = nc.dram_tensor("x", [ROWS, N], f32, kind="ExternalInput")
    # w arrives host-pre-broadcast across partitions: one clean DMA, no
    # cross-engine broadcast chain for every consumer to wait on
    w_d = nc.dram_tensor("w", [128, L], f32, kind="ExternalInput")
    # [b, 3, C, L] has the same memory layout as the final [b, 3C, L]
    o_d = nc.dram_tensor("out", [BPC, 3, C, L], f32, kind="ExternalOutput")

    chunks = _chunks()

    with TileContext(nc) as tc:
        with (
            tc.tile_pool(name="xinp", bufs=3) as xinp,
            tc.tile_pool(name="io", bufs=2) as iop,
            tc.tile_pool(name="work", bufs=2) as wkp,
        ):
            for _rep in range(n_repeat):
              for l0, l1 in chunks:
                # per-chunk slice of w, shared by both row-tiles
                w_sb = wkp.tile([128, CKM], f32, tag="w_sb")
                nc.gpsimd.dma_start(out=w_sb[:, : l1 - l0], in_=w_d[:, l0:l1])
                for rt in range(RT):
                    r0 = rt * 128
                    ck = l1 - l0
                    pieces = _pieces_for_chunk(s_arr, l0, l1)
                    g0 = 2 * l0 - 4  # x~ index of xin column 0
                    hi = max(
                        2 * l1 - 2,
                        max(2 * (b - 1) - s + 1 for (a, b, s) in pieces),
                    )
                    width = hi - g0 + 1
                    nv_l = max(0, -g0)  # left virtual cols
                    nv_r = max(0, hi - (N - 1))  # right virtual cols
                    real_w = width - nv_l - nv_r

                    xin = xinp.tile([128, 2 * CKM + 4], f32, tag="xin")
                    # SWDGE spreads one transfer across all 16 SDMA engines;
                    # the HWDGE rings here land it on ~2 engines only
                    nc.gpsimd.dma_start(
                        out=xin[:, nv_l : nv_l + real_w],
                        in_=x_d[r0 : r0 + 128, g0 + nv_l : g0 + nv_l + real_w],
                    )
                    if nv_l:
                        # x~[g0+j] = x[-(g0+j)-1] -> src col (-2*g0 - 1) - j
                        sc = -2 * g0 - 1
                        nc.vector.tensor_copy(
                            xin[:, 0:nv_l], xin[:, sc : sc - nv_l : -1]
                        )
                    if nv_r:
                        # x~[g0+j] = x[2N-1-g0-j] -> src col (2N-1-2*g0) - j
                        j0v = width - nv_r
                        sc = 2 * N - 1 - 2 * g0 - j0v
                        nc.vector.tensor_copy(
                            xin[:, j0v:width], xin[:, sc : sc - nv_r : -1]
                        )

                    # ---- lifting DWT: D2 = E1 - 2*O1, then D2 += E2
                    # (in place; saves the separate sE tile)
                    cnt = ck + 1  # D2 computed for l in [l0-1, l1)
                    D2 = wkp.tile([128, CKM + 1], f32, tag="D2")
                    nc.vector.scalar_tensor_tensor(
                        D2[:, :cnt],
                        xin[:, 1 : 1 + 2 * cnt : 2],
                        -2.0,
                        xin[:, 0 : 2 * cnt : 2],
                        op0=MUL,
                        op1=ADD,
                    )
                    nc.vector.tensor_add(
                        D2[:, :cnt],
                        D2[:, :cnt],
                        xin[:, 2 : 2 + 2 * cnt : 2],
                    )
                    d_t = iop.tile([128, CKM], f32, tag="d_t")
                    nc.scalar.mul(d_t[:, :ck], D2[:, 1 : 1 + ck], ISQ8)
                    sD2 = wkp.tile([128, CKM], f32, tag="sD2")
                    nc.gpsimd.tensor_add(
                        sD2[:, :ck], D2[:, 0:ck], D2[:, 1 : 1 + ck]
                    )
                    # A written straight into a_t, then scaled in place
                    a_t = iop.tile([128, CKM], f32, tag="a_t")
                    nc.vector.scalar_tensor_tensor(
                        a_t[:, :ck],
                        sD2[:, :ck],
                        -0.125,
                        xin[:, 2 : 2 + 2 * ck : 2],
                        op0=MUL,
                        op1=ADD,
                    )
                    nc.scalar.mul(a_t[:, :ck], a_t[:, :ck], SQ2)

                    # ---- linear interp, computed in place in ds_t:
                    # ds = (x_i1 - x_i0); ds *= w; ds += x_i0
                    ds_t = iop.tile([128, CKM], f32, tag="ds_t")
                    for a, b, s in pieces:
                        pc = b - a
                        c0 = a - l0
                        i0c = 2 * a - s - g0
                        nc.vector.tensor_sub(
                            ds_t[:, c0 : c0 + pc],
                            xin[:, i0c + 1 : i0c + 1 + 2 * pc : 2],
                            xin[:, i0c : i0c + 2 * pc : 2],
                        )
                        nc.gpsimd.tensor_mul(
                            ds_t[:, c0 : c0 + pc],
                            ds_t[:, c0 : c0 + pc],
                            w_sb[:, a - l0 : b - l0],
                        )
                        nc.vector.tensor_add(
                            ds_t[:, c0 : c0 + pc],
                            ds_t[:, c0 : c0 + pc],
                            xin[:, i0c : i0c + 2 * pc : 2],
                        )

                    # ---- store: clean 3-dim DMAs via SWDGE
                    for sec, t in ((0, ds_t), (1, a_t), (2, d_t)):
                        nc.gpsimd.dma_start(
                            out=o_d[2 * rt : 2 * rt + 2, sec, :, l0:l1],
                            in_=t[:, :ck],
                        )

    return nc, w32


_prog_cache = {}


def kernel(x):
    import numpy as np  # noqa: F811

    from concourse.bass_utils import run_bass_kernel_spmd

    x = np.asarray(x, dtype=np.float32)
    assert x.shape == (B, C, N), x.shape

    if "nc" not in _prog_cache:
        _prog_cache["nc"], _prog_cache["w32"] = _build_program()
    nc = _prog_cache["nc"]
    w32 = np.ascontiguousarray(
        np.broadcast_to(_prog_cache["w32"].reshape(1, L), (128, L))
    )

    in_maps = [
        {"x": np.ascontiguousarray(x[c * BPC : (c + 1) * BPC].reshape(ROWS, N)),
         "w": w32}
        for c in range(NCORES)
    ]
    res = run_bass_kernel_spmd(nc, in_maps, core_ids=list(range(NCORES)))
    out = np.concatenate(
        [r["out"].reshape(BPC, 3 * C, L) for r in res.results], axis=0
    )
    assert out.shape == (B, 3 * C, L)
    return out


# revision 23
# speedup vs baseline: 1.2434x; 1.2434x over previous
"""Trainium2 Bass kernel for nn_DWT1D: single-level bior2.2 DWT (symmetric
mode, pywt convention) + linear downsample, fused.

    x [32, 64, 16384] f32  ->  out [32, 192, 8194] f32
    out[:, 0:64]    = linear interp of x to length L   (align_corners=False)
    out[:, 64:128]  = DWT approx coeffs
    out[:, 128:192] = DWT detail coeffs

Strategy: pure data parallel over batch (4 batches = 256 (b,c) rows per
NeuronCore x 8 cores).  Rows live on SBUF partitions (2 row-tiles of 128);
the signal axis is chunked.  The bior2.2 (CDF 5/3) filters are evaluated
with a lifting-style factorization on even/odd phases, read directly from
the interleaved input via stride-2 APs (fp32 tensor_tensor is 1x mode on
DVE regardless of stride, so deinterleaving would be pure overhead):

    D2[l] = x[2l-2] + x[2l] - 2*x[2l-1]         (DVE: TT add + STT)
    d[l]  = 0.35355339 * D2[l]                  (ACT scaled copy)
    A[l]  = x[2l-2] - 0.125*(D2[l-1] + D2[l])   (Pool add + DVE STT)
    a[l]  = sqrt(2) * A[l]                      (ACT scaled copy)

The interp is piecewise "i0 = 2l - s" with s constant on 6 runs; weights
are host-precomputed replicating the reference's f32 arithmetic exactly,
shipped as a [1, L] input and broadcast across partitions once via a K=1
PE matmul (ones^T @ w -> PSUM -> SBUF).

    ds[l] = x[i0] + w[l]*(x[i1] - x[i0])        (DVE sub + Pool mul + DVE add)

Symmetric-extension halos are materialized in the input tile with small
reversed-AP on-chip copies so every output column uses the interior
formula.  All DMAs are HWDGE (input on SP ring, output on ACT ring).
"""

import json

import numpy as np

# ---------------------------------------------------------------- constants
B, C, N = 32, 64, 16384
L = (N + 6 - 1) // 2  # 8194
NCORES = 8
BPC = B // NCORES  # batches per core
ROWS = BPC * C  # 256 rows per core
RT = ROWS // 128  # row-tiles per core
CK = 2048  # output-chunk length (last chunk 2050)
CKM = L - (L // CK - 1) * CK if L % CK else CK  # max chunk len
SQ2 = 1.4142135623730951
ISQ8 = 0.35355339059327378  # 1/(2*sqrt(2))

_MAX_WAITS = 1
_REAL_ENGINES = {"SP", "DVE", "PE", "Pool", "Activation"}


# ------------------------------------------------- BIR sync-wait splitting
def _split_sync_waits(bir_bytes, max_waits=_MAX_WAITS):
    """This neuronxcc build rejects instructions carrying more than one
    semaphore wait ("Too many sync wait commands").  Move excess waits onto
    same-engine NoOps inserted right before the instruction — an earlier
    wait on the same in-order engine is semantically identical."""
    d = json.loads(bir_bytes)
    ctr = 0
    changed = False
    for fn in d.get("functions", []):
        for bb in fn.get("blocks", []):
            out = []
            for inst in bb.get("instructions", []):
                si = inst.get("sync_info")
                ow = (si or {}).get("on_wait") or []
                if len(ow) > max_waits and inst.get("engine") in _REAL_ENGINES:
                    si["on_wait"] = ow[:max_waits]
                    for w in ow[max_waits:]:
                        ctr += 1
                        changed = True
                        out.append(
                            {
                                "debug": inst.get("debug", 0),
                                "engine": inst["engine"],
                                "ins": [],
                                "outs": [],
                                "name": f"{inst['name']}-wsp{ctr}",
                                "opcode": "NoOp",
                                "sync_info": {"on_update": [], "on_wait": [w]},
                            }
                        )
                out.append(inst)
            bb["instructions"] = out
    if not changed:
        return bir_bytes
    return json.dumps(d).encode()


_wait_patch_done = False


def _install_wait_splitter():
    global _wait_patch_done
    if _wait_patch_done:
        return
    _wait_patch_done = True
    import concourse.bass as bass

    orig = bass.Bass.to_json_bytes

    def to_json_bytes(self, *a, **k):
        return _split_sync_waits(orig(self, *a, **k))

    bass.Bass.to_json_bytes = to_json_bytes


# ------------------------------------------------------- interp host consts
def _interp_consts():
    """Replicate the reference's f32 arithmetic for the interp grid exactly:
    src = (arange(L) + 0.5) * f32(N/L) - 0.5 ; i0 = floor(src); w = src-i0."""
    ratio = np.float32(N / L)
    src = (np.arange(L, dtype=np.float32) + np.float32(0.5)) * ratio
    src = src - np.float32(0.5)
    src = np.clip(src, np.float32(0.0), np.float32(N - 1))
    i0 = np.floor(src).astype(np.int64)
    w = src - i0.astype(np.float32)
    ls = np.arange(L, dtype=np.int64)
    s = 2 * ls - i0
    # interior-formula preconditions (no clipping active, i1 = i0+1 in range)
    assert i0.min() >= 0 and i0.max() <= N - 2, (i0.min(), i0.max())
    assert s.min() >= 0
    return w.astype(np.float32), s


def _pieces_for_chunk(s, l0, l1):
    """Split [l0, l1) into maximal runs of constant s."""
    pieces = []
    a = l0
    while a < l1:
        b = a + 1
        while b < l1 and s[b] == s[a]:
            b += 1
        pieces.append((a, b, int(s[a])))
        a = b
    return pieces


def _chunks():
    starts = list(range(0, L - CKM + 1, CK))
    out = []
    for i, st in enumerate(starts):
        en = st + CK if i < len(starts) - 1 else L
        out.append((st, en))
    assert out[-1][1] == L
    return out


# ------------------------------------------------------------ program build
def _build_program(n_repeat=1):
    import concourse.bass as bass
    import concourse.mybir as mybir
    from concourse.tile import TileContext

    _install_wait_splitter()

    f32 = mybir.dt.float32
    MUL = mybir.AluOpType.mult
    ADD = mybir.AluOpType.add

    w32, s_arr = _interp_consts()

    nc = bass.Bass(dynamic_dma_scratch_size=32768)
    x_d = nc.dram_tensor("x", [ROWS, N], f32, kind="ExternalInput")
    # w arrives host-pre-broadcast across partitions: one clean DMA, no
    # cross-engine broadcast chain for every consumer to wait on
    w_d = nc.dram_tensor("w", [128, L], f32, kind="ExternalInput")
    # [b, 3, C, L] has the same memory layout as the final [b, 3C, L]
    o_d = nc.dram_tensor("out", [BPC, 3, C, L], f32, kind="ExternalOutput")

    chunks = _chunks()

    with TileContext(nc) as tc:
        with (
            tc.tile_pool(name="xinp", bufs=3) as xinp,
            tc.tile_pool(name="io", bufs=2) as iop,
            tc.tile_pool(name="work", bufs=2) as wkp,
        ):
            def chunk_geom(l0, l1):
                pieces = _pieces_for_chunk(s_arr, l0, l1)
                g0 = 2 * l0 - 4  # x~ index of xin column 0
                hi = max(
                    2 * l1 - 2,
                    max(2 * (b - 1) - s + 1 for (a, b, s) in pieces),
                )
                width = hi - g0 + 1
                nv_l = max(0, -g0)  # left virtual cols
                nv_r = max(0, hi - (N - 1))  # right virtual cols
                return pieces, g0, width, nv_l, nv_r

            def issue_in_dma(l0, l1, rt):
                # SWDGE spreads one transfer across all 16 SDMA engines;
                # the HWDGE rings here land it on ~2 engines only
                _, g0, width, nv_l, nv_r = chunk_geom(l0, l1)
                real_w = width - nv_l - nv_r
                xin = xinp.tile([128, 2 * CKM + 4], f32, tag="xin")
                r0 = rt * 128
                nc.gpsimd.dma_start(
                    out=xin[:, nv_l : nv_l + real_w],
                    in_=x_d[r0 : r0 + 128, g0 + nv_l : g0 + nv_l + real_w],
                )
                return xin

            for _rep in range(n_repeat):
              insts = [(l0, l1, rt) for l0, l1 in chunks for rt in range(RT)]
              # prefetch: issue each input DMA one iteration ahead so the
              # SWDGE FIFO serves it before the previous iteration's outputs
              pending = [issue_in_dma(*insts[0])]
              w_cur = None
              for idx, (l0, l1, rt) in enumerate(insts):
                    ck = l1 - l0
                    if rt == 0:
                        w_cur = wkp.tile([128, CKM], f32, tag="w_sb")
                        nc.gpsimd.dma_start(
                            out=w_cur[:, :ck], in_=w_d[:, l0:l1]
                        )
                    w_sb = w_cur
                    if idx + 1 < len(insts):
                        pending.append(issue_in_dma(*insts[idx + 1]))
                    xin = pending.pop(0)
                    pieces, g0, width, nv_l, nv_r = chunk_geom(l0, l1)
                    if nv_l:
                        # x~[g0+j] = x[-(g0+j)-1] -> src col (-2*g0 - 1) - j
                        sc = -2 * g0 - 1
                        nc.vector.tensor_copy(
                            xin[:, 0:nv_l], xin[:, sc : sc - nv_l : -1]
                        )
                    if nv_r:
                        # x~[g0+j] = x[2N-1-g0-j] -> src col (2N-1-2*g0) - j
                        j0v = width - nv_r
                        sc = 2 * N - 1 - 2 * g0 - j0v
                        nc.vector.tensor_copy(
                            xin[:, j0v:width], xin[:, sc : sc - nv_r : -1]
                        )

                    # ---- lifting DWT: D2 = E1 - 2*O1, then D2 += E2
                    # (in place; saves the separate sE tile)
                    cnt = ck + 1  # D2 computed for l in [l0-1, l1)
                    D2 = wkp.tile([128, CKM + 1], f32, tag="D2")
                    nc.vector.scalar_tensor_tensor(
                        D2[:, :cnt],
                        xin[:, 1 : 1 + 2 * cnt : 2],
                        -2.0,
                        xin[:, 0 : 2 * cnt : 2],
                        op0=MUL,
                        op1=ADD,
                    )
                    nc.vector.tensor_add(
                        D2[:, :cnt],
                        D2[:, :cnt],
                        xin[:, 2 : 2 + 2 * cnt : 2],
                    )
                    d_t = iop.tile([128, CKM], f32, tag="d_t")
                    nc.scalar.mul(d_t[:, :ck], D2[:, 1 : 1 + ck], ISQ8)
                    sD2 = wkp.tile([128, CKM], f32, tag="sD2")
                    nc.gpsimd.tensor_add(
                        sD2[:, :ck], D2[:, 0:ck], D2[:, 1 : 1 + ck]
                    )
                    # A written straight into a_t, then scaled in place
                    a_t = iop.tile([128, CKM], f32, tag="a_t")
                    nc.vector.scalar_tensor_tensor(
                        a_t[:, :ck],
                        sD2[:, :ck],
                        -0.125,
                        xin[:, 2 : 2 + 2 * ck : 2],
                        op0=MUL,
                        op1=ADD,
                    )
                    nc.scalar.mul(a_t[:, :ck], a_t[:, :ck], SQ2)

                    # ---- linear interp, computed in place in ds_t:
                    # ds = (x_i1 - x_i0); ds *= w; ds += x_i0
                    ds_t = iop.tile([128, CKM], f32, tag="ds_t")
                    for a, b, s in pieces:
                        pc = b - a
                        c0 = a - l0
                        i0c = 2 * a - s - g0
                        nc.vector.tensor_sub(
                            ds_t[:, c0 : c0 + pc],
                            xin[:, i0c + 1 : i0c + 1 + 2 * pc : 2],
                            xin[:, i0c : i0c + 2 * pc : 2],
                        )
                        nc.gpsimd.tensor_mul(
                            ds_t[:, c0 : c0 + pc],
                            ds_t[:, c0 : c0 + pc],
                            w_sb[:, a - l0 : b - l0],
                        )
                        nc.vector.tensor_add(
                            ds_t[:, c0 : c0 + pc],
                            ds_t[:, c0 : c0 + pc],
                            xin[:, i0c : i0c + 2 * pc : 2],
                        )

                    # ---- store: clean 3-dim DMAs via SWDGE
                    for sec, t in ((0, ds_t), (1, a_t), (2, d_t)):
                        nc.gpsimd.dma_start(
                            out=o_d[2 * rt : 2 * rt + 2, sec, :, l0:l1],
                            in_=t[:, :ck],
                        )

    return nc, w32


_prog_cache = {}


def kernel(x):
    import numpy as np  # noqa: F811

    from concourse.bass_utils import run_bass_kernel_spmd

    x = np.asarray(x, dtype=np.float32)
    assert x.shape == (B, C, N), x.shape

    if "nc" not in _prog_cache:
        _prog_cache["nc"], _prog_cache["w32"] = _build_program()
    nc = _prog_cache["nc"]
    w32 = np.ascontiguousarray(
        np.broadcast_to(_prog_cache["w32"].reshape(1, L), (128, L))
    )

    in_maps = [
        {"x": np.ascontiguousarray(x[c * BPC : (c + 1) * BPC].reshape(ROWS, N)),
         "w": w32}
        for c in range(NCORES)
    ]
    res = run_bass_kernel_spmd(nc, in_maps, core_ids=list(range(NCORES)))
    out = np.concatenate(
        [r["out"].reshape(BPC, 3 * C, L) for r in res.results], axis=0
    )
    assert out.shape == (B, 3 * C, L)
    return out
